# revision 25
# baseline (speedup 1.0000x reference)
"""Galerkin-attention encoder block on 8 TRN2 NeuronCores.

Sharding: tokens (N=8192 -> 1024/core). The only cross-core dependency is
the Galerkin contraction scores[b,h] = sum_n k[n] (x) v[n] / N, reduced with
four per-batch bf16 AllReduces that overlap local compute.

All device compute runs in "transposed space" (features on partitions,
tokens on the free axis) against host-side pre-transposed x^T, so the
kernel needs no on-device transposes anywhere.

Precision/speed scheme: every large matmul (QKV projections, FFN1, FFN2)
runs in fp8e4m3 with DoubleRow perf mode (2 contraction chunks per
instruction). Weights are pre-scaled by 32 into fp8's normal range; the
1/32 descale folds into the activation-scale of the op that drains each
PSUM accumulation. K/V feed LayerNorm, which is scale-invariant, so their
descale is free. The fp8 noise of the FFN is tamed by routing the linear
bulk of SiLU through a precise bf16 bypass:
  silu(z) = z/2 + g(z),  g(z) = (z/2)*tanh(z/2)
  ffn_out = x1 @ (0.5*W1@W2) [bf16, D x D]  +  g(z) @ W2 [fp8]
W12 = 0.5*W1@W2 is computed host-side; the bypass matmuls accumulate into
the same PSUM group as the fp8 FFN2 matmuls. The residual stream x, x1
stays bf16 end to end.

Schedule notes (vs the first working version):
- Startup: only the tensors the first matmuls need (x super-tile 0, Wk,
  Wv) are DMA'd ahead of the 11MB of phase-B weights, which previously
  monopolized HBM for ~50us before the first matmul could issue.
- Phase A LN runs batched: one segmented bn_stats per K/V projection half
  (4 heads at once) and all mean/var combine arithmetic on [128,16] tiles,
  quartering DVE instruction count (DVE was phase A's bottleneck at 96%).
- The scores AllReduce runs in bf16 (error-neutral, verified in sim) so
  the reduced scores DMA straight into their SBUF tile with no f32
  staging + cast copy.
- The last two subtiles' scores matmuls of each batch are deferred past
  the next batch's first projections, so the batch boundary no longer
  stalls the PE on the LayerNorm tail.
"""

import numpy as np
import ml_dtypes

B, N, D = 4, 8192, 1024
H, DK = 8, 128
FF = 4096
EPS = 1e-5
N_CORES = 8
NT = N // N_CORES          # tokens per core
KC = D // 128              # feature chunks of 128
FC = FF // 128
SUP = 512                  # tokens per super-tile in phase B
NSUP = NT // SUP
SUB = 128                  # tokens per sub-tile in phase A
NSUB = SUP // SUB
SW = 32.0                  # fp8 weight pre-scale

_GRAPH_CACHE = {}


def _build(flags):
    import concourse.bass as bass
    import concourse.tile as tile
    from concourse import bacc, mybir
    from contextlib import ExitStack

    has_bk, has_bv, has_b1, has_b2, has_affine = flags
    f32 = mybir.dt.float32
    bf16 = mybir.dt.bfloat16
    f8 = mybir.dt.float8e4
    DR = mybir.MatmulPerfMode.DoubleRow

    nc = bacc.Bacc("TRN2", target_bir_lowering=False, debug=False,
                   num_devices=N_CORES)

    # All tensors arrive pre-tiled in their exact SBUF layouts so every DMA
    # is a contiguous-per-partition copy.
    xTb_d = nc.dram_tensor("xTb", [B, NSUP, 128, KC, SUP], f8, kind="ExternalInput")
    xTbBf_d = nc.dram_tensor("xTbBf", [B, NSUP, 128, KC, SUP], bf16, kind="ExternalInput")
    delta_d = nc.dram_tensor("delta", [128, NT // 128], f32, kind="ExternalInput")
    wq_d = nc.dram_tensor("Wq", [128, KC, D], f8, kind="ExternalInput")
    wk_d = nc.dram_tensor("Wk", [128, KC, D], f8, kind="ExternalInput")
    wv_d = nc.dram_tensor("Wv", [128, KC, D], f8, kind="ExternalInput")
    w1_d = nc.dram_tensor("W1", [128, KC, FF], f8, kind="ExternalInput")
    w2_d = nc.dram_tensor("W2", [128, FC, D], f8, kind="ExternalInput")
    w12_d = nc.dram_tensor("W12", [128, KC, D], bf16, kind="ExternalInput")
    bq_d = nc.dram_tensor("bq", [128, KC], f32, kind="ExternalInput")
    b1_d = nc.dram_tensor("b1", [128, FC], f32, kind="ExternalInput")
    b1h_d = nc.dram_tensor("b1h", [128, FC], f32, kind="ExternalInput") if has_b1 else None
    bk_d = nc.dram_tensor("bk", [D], f32, kind="ExternalInput") if has_bk else None
    bv_d = nc.dram_tensor("bv", [D], f32, kind="ExternalInput") if has_bv else None
    b2_d = nc.dram_tensor("b2", [128, KC], f32, kind="ExternalInput") if has_b2 else None
    gamma_d = nc.dram_tensor("gamma", [D], f32, kind="ExternalInput") if has_affine else None
    beta_d = nc.dram_tensor("beta", [D], f32, kind="ExternalInput") if has_affine else None
    out_d = nc.dram_tensor("outT", [B, NSUP, 128, KC, SUP], bf16, kind="ExternalOutput")

    # Super-tiles whose Q projection runs inside phase A (filling its PE
    # bubbles) with the result staged through a DRAM scratch. (0,0) stays in
    # phase B: its window races the Wq load at startup.
    Q_IN_A = {(0, 1), (1, 0), (1, 1)}

    sub_ = mybir.AluOpType.subtract
    mult = mybir.AluOpType.mult
    add_ = mybir.AluOpType.add
    ACT = mybir.ActivationFunctionType

    with tile.TileContext(nc) as tc, ExitStack() as ctx:
        singles = ctx.enter_context(tc.tile_pool(name="singles", bufs=1))
        dram = ctx.enter_context(tc.tile_pool(name="dram", bufs=1, space="DRAM"))

        eps_t = singles.tile([128, 1], f32)
        nc.vector.memset(eps_t, EPS)
        delta_sb = singles.tile([128, NT // 128], f32)
        scores_bf = singles.tile([128, B, H, DK], bf16)

        cc_in = dram.tile([B, 128, H, DK], bf16)
        cc_out = [dram.tile([128, H, DK], bf16, addr_space="Shared",
                            name=f"cc_out{b}") for b in range(B)]

        # Weight pools span both phases; DMAs are issued inside phase A
        # AFTER the startup-critical loads so they don't starve them.
        w_b2a_cm = tc.tile_pool(name="w_b2a", bufs=1)
        w_b2a = w_b2a_cm.__enter__()
        w2_sb = w_b2a.tile([128, FC, D], f8)
        w12_sb = w_b2a.tile([128, KC, D], bf16)
        w_ab1_cm = tc.tile_pool(name="w_ab1", bufs=1)
        w_ab1 = w_ab1_cm.__enter__()
        wq_sb = w_ab1.tile([128, KC, D], f8)
        w1_sb = w_ab1.tile([128, KC, FF], f8)
        bq_sb = w_ab1.tile([128, KC], f32)
        b1_sb = w_ab1.tile([128, FC], f32)

        # Phase-B x pools opened early so (b=0, s=0) can prefetch during
        # phase A; the fp8 tile doubles as phase A's first x super-tile.
        b_x_cm = tc.tile_pool(name="b_x", bufs=2)
        b_x = b_x_cm.__enter__()
        b_xb_cm = tc.tile_pool(name="b_xb", bufs=2)
        b_xb = b_xb_cm.__enter__()

        # ---------------- Phase A: k, v, LN, partial scores, AllReduce ----
        with (
            tc.tile_pool(name="wa", bufs=1) as wa,
            tc.tile_pool(name="a_x", bufs=2) as a_x,
            tc.tile_pool(name="a_kvf", bufs=10) as a_kvf,
            tc.tile_pool(name="a_kvb", bufs=4) as a_kvb,
            tc.tile_pool(name="a_ln", bufs=8) as a_ln,
            tc.tile_pool(name="a_sc", bufs=2) as a_sc,
            tc.tile_pool(name="a_qt", bufs=2) as a_qt,
            tc.tile_pool(name="a_kvps", bufs=5, space="PSUM") as a_kvps,
            tc.tile_pool(name="a_sps", bufs=1, space="PSUM") as a_sps,
            tc.tile_pool(name="a_qps", bufs=1, space="PSUM") as a_qps,
        ):
            qt_dr = {bs: dram.tile([128, KC, SUP], bf16, name=f"qtd{bs[0]}{bs[1]}")
                     for bs in Q_IN_A}
            # Startup-critical DMAs first, all on the sync queue except Wv
            # (gpsimd) so the two halves land in parallel.
            xtb00 = b_x.tile([128, KC, SUP], f8, tag="xtb1")
            nc.sync.dma_start(out=xtb00[:], in_=xTb_d.ap()[0, 0])
            wk_sb = wa.tile([128, KC, D], f8)
            nc.sync.dma_start(out=wk_sb[:, :, 0:512], in_=wk_d.ap()[:, :, 0:512])
            nc.sync.dma_start(out=wk_sb[:, :, 512:D], in_=wk_d.ap()[:, :, 512:D])
            wv_sb = wa.tile([128, KC, D], f8)
            nc.gpsimd.dma_start(out=wv_sb[:, :, 0:512], in_=wv_d.ap()[:, :, 0:512])
            nc.gpsimd.dma_start(out=wv_sb[:, :, 512:D], in_=wv_d.ap()[:, :, 512:D])
            nc.sync.dma_start(out=delta_sb[:], in_=delta_d.ap())
            xtbb00 = b_xb.tile([128, KC, SUP], bf16, tag="xtbb")
            nc.sync.dma_start(out=xtbb00[:], in_=xTbBf_d.ap()[0, 0])
            # Bulk phase-B weights stream behind the critical loads on the
            # same (sync) queue; phase A x tiles ride gpsimd so they never
            # queue behind this 11MB.
            nc.sync.dma_start(out=wq_sb[:], in_=wq_d.ap())
            nc.sync.dma_start(out=w1_sb[:], in_=w1_d.ap())
            nc.sync.dma_start(out=w2_sb[:], in_=w2_d.ap())
            nc.sync.dma_start(out=w12_sb[:], in_=w12_d.ap())
            nc.scalar.dma_start(out=bq_sb[:], in_=bq_d.ap())
            nc.scalar.dma_start(out=b1_sb[:], in_=b1_d.ap())
            if has_b1:
                b1h_sb = w_ab1.tile([128, FC], f32)
                nc.scalar.dma_start(out=b1h_sb[:], in_=b1h_d.ap())
            if has_b2:
                b2_sb = w_ab1.tile([128, KC], f32)
                nc.scalar.dma_start(out=b2_sb[:], in_=b2_d.ap())
            if has_affine:
                gamma_sb = w_ab1.tile([128, D], f32)
                nc.scalar.dma_start(out=gamma_sb[:], in_=gamma_d.ap().to_broadcast([128, D]))
                beta_sb = w_ab1.tile([128, D], f32)
                nc.scalar.dma_start(out=beta_sb[:], in_=beta_d.ap().to_broadcast([128, D]))
            if has_bk:
                bk_sb = w_ab1.tile([128, 2, 4, 128], f32)
                nc.scalar.dma_start(out=bk_sb[:], in_=bk_d.ap().to_broadcast([128, D]))
            if has_bv:
                bv_sb = w_ab1.tile([128, 2, 4, 128], f32)
                nc.scalar.dma_start(out=bv_sb[:], in_=bv_d.ap().to_broadcast([128, D]))

            def drain_scores(b, scores_ps):
                # f32 PSUM -> bf16 staging -> HBM -> bf16 AllReduce. The
                # reduced scores later DMA straight into scores_bf.
                sc_sb = a_sc.tile([128, H, DK], bf16, tag="scsb")
                nc.scalar.activation(out=sc_sb[:], in_=scores_ps[:, :, :],
                                     func=ACT.Copy)
                nc.sync.dma_start(out=cc_in[b], in_=sc_sb[:])
                nc.gpsimd.collective_compute(
                    "AllReduce", mybir.AluOpType.add,
                    replica_groups=[list(range(N_CORES))],
                    ins=[cc_in[b].opt()], outs=[cc_out[b].opt()])

            def emit_scores(item):
                b, scores_ps, halves, last = item
                for oc in range(2):
                    kf, vf = halves[oc]
                    for h in range(4):
                        ph = slice(h * DK, (h + 1) * DK)
                        nc.tensor.matmul(
                            scores_ps[:, oc * 4 + h, :], lhsT=kf[:, ph], rhs=vf[:, ph],
                            start=False, stop=last, skip_group_check=True)
                if last:
                    drain_scores(b, scores_ps)

            pendq = []
            for b in range(B):
                scores_ps = a_sps.tile([128, H, DK], f32, tag="scores")
                # Four per-head accumulation groups share each PSUM bank, and
                # a start=True matmul clears its whole bank - zero once and
                # accumulate with start=False instead.
                nc.vector.memset(scores_ps, 0.0)

                for s in range(NSUP):
                    if b == 0 and s == 0:
                        xtb = xtb00
                    else:
                        xtb = a_x.tile([128, KC, SUP], f8, tag="xtb")
                        nc.gpsimd.dma_start(out=xtb[:], in_=xTb_d.ap()[b, s])
                    q_in_a = (b, s) in Q_IN_A
                    if q_in_a:
                        qt_stage = a_qt.tile([128, KC, SUP], bf16, tag="qstage")

                    def emit_q(qm):
                        # One Q output chunk for this super-tile, filling PE
                        # bubbles left by the DVE-bound LN chain. Numerics
                        # are identical to the phase-B version.
                        q_ps = a_qps.tile([128, SUP], f32, tag="qps")
                        for kc in range(0, KC, 2):
                            nc.tensor.matmul(
                                q_ps[:],
                                lhsT=wq_sb[:, kc:kc + 2, qm * 128:(qm + 1) * 128],
                                rhs=xtb[:, kc:kc + 2, :],
                                start=(kc == 0), stop=(kc == KC - 2),
                                perf_mode=DR)
                        nc.scalar.activation(out=qt_stage[:, qm, :], in_=q_ps[:],
                                             func=ACT.Identity,
                                             bias=bq_sb[:, qm:qm + 1],
                                             scale=1.0 / SW)

                    for sb in range(NSUB):
                        gsub = s * NSUB + sb
                        tsl = bass.ts(sb, SUB)

                        def half_proj(w_sb, bias_sb, oc, tag):
                            # One 512-feature half of a K/V projection,
                            # shaped [tok, head, dk] for segmented bn_stats.
                            ps = a_kvps.tile([128, 4, DK], f32, tag="kv")
                            for kc in range(0, KC, 2):
                                nc.tensor.matmul(
                                    ps[:, :, :],
                                    lhsT=xtb[:, kc:kc + 2, tsl],
                                    rhs=w_sb[:, kc:kc + 2, oc * 512:(oc + 1) * 512],
                                    start=(kc == 0), stop=(kc == KC - 2),
                                    perf_mode=DR)
                            if bias_sb is not None:
                                nc.vector.tensor_add(ps[:, :, :], ps[:, :, :],
                                                     bias_sb[:, oc])
                            return ps

                        k_ps = [half_proj(wk_sb, bk_sb if has_bk else None, oc, "k")
                                for oc in range(2)]
                        if q_in_a:
                            emit_q(2 * sb)
                        v_ps = [half_proj(wv_sb, bv_sb if has_bv else None, oc, "v")
                                for oc in range(2)]
                        if q_in_a:
                            emit_q(2 * sb + 1)

                        # Drain PSUM to SBUF bf16 immediately: the PSUM banks
                        # recycle after one op instead of after the whole LN
                        # chain, so the PE never stalls on bank pressure (a
                        # stall >3.4us re-throttles the PE clock). LN then
                        # runs entirely from SBUF at 16-bit DVE speed.
                        kb = []
                        vb = []
                        for oc in range(2):
                            kbt = a_kvb.tile([128, 4, DK], bf16, tag="kb")
                            nc.scalar.activation(out=kbt[:, :, :],
                                                 in_=k_ps[oc][:, :, :],
                                                 func=ACT.Copy)
                            kb.append(kbt)
                            vbt = a_kvb.tile([128, 4, DK], bf16, tag="vb")
                            nc.vector.tensor_copy(out=vbt[:, :, :],
                                                  in_=v_ps[oc][:, :, :])
                            vb.append(vbt)

                        # Per-head stats into one batched tile; stat columns
                        # are [k0 h0-3 | k1 h0-3 | v0 h0-3 | v1 h0-3] so the
                        # delta fold below is a single [128,8] op.
                        stats = a_ln.tile([128, 16, 6], f32, tag="stats")
                        for oc in range(2):
                            for h in range(4):
                                nc.vector.bn_stats(out=stats[:, 4 * oc + h, :],
                                                   in_=kb[oc][:, h, :])
                                nc.vector.bn_stats(out=stats[:, 8 + 4 * oc + h, :],
                                                   in_=vb[oc][:, h, :])
                        # Combine bn_stats' even/odd halves with batched
                        # [128,16] arithmetic:
                        # M2 = M2e + M2o + (mu_e - mu_o)^2 * (DK/4),
                        # var = M2/DK, mu2 = mu_e + mu_o (= 2*mu).
                        dmu = a_ln.tile([128, 16], f32, tag="dmu")
                        nc.vector.scalar_tensor_tensor(
                            out=dmu[:], in0=stats[:, :, 1], scalar=1.0,
                            in1=stats[:, :, 4], op0=mult, op1=sub_)
                        m2 = a_ln.tile([128, 16], f32, tag="m2")
                        nc.vector.scalar_tensor_tensor(
                            out=m2[:], in0=stats[:, :, 2], scalar=1.0,
                            in1=stats[:, :, 5], op0=mult, op1=add_)
                        dmu2 = a_ln.tile([128, 16], f32, tag="dmu2")
                        nc.vector.tensor_mul(dmu2[:], dmu[:], dmu[:])
                        m2t = a_ln.tile([128, 16], f32, tag="m2t")
                        nc.vector.scalar_tensor_tensor(
                            out=m2t[:], in0=dmu2[:], scalar=float(DK) / 4.0,
                            in1=m2[:], op0=mult, op1=add_)
                        mu2 = a_ln.tile([128, 16], f32, tag="mu2")
                        nc.vector.scalar_tensor_tensor(
                            out=mu2[:], in0=stats[:, :, 1], scalar=1.0,
                            in1=stats[:, :, 4], op0=mult, op1=add_)
                        # rstd = 1/sqrt(var+eps) in one ACT (var >= 0 so the
                        # abs is a no-op; fp8 noise dwarfs the table error)
                        rstd = a_ln.tile([128, 16], f32, tag="rstd")
                        nc.scalar.activation(out=rstd[:], in_=m2t[:],
                                             func=ACT.Abs_reciprocal_sqrt,
                                             bias=eps_t[:], scale=1.0 / DK)
                        if not has_affine:
                            # delta/N folds into K's rstd (columns 0-7).
                            # (stays on DVE: Pool lacks TensorScalarPtr)
                            nc.vector.tensor_scalar_mul(
                                out=rstd[:, 0:8], in0=rstd[:, 0:8],
                                scalar1=delta_sb[:, gsub:gsub + 1])
                        # nmr = -mu * rstd  (= (mu2 * -0.5) * rstd)
                        nmr = a_ln.tile([128, 16], f32, tag="nmr")
                        nc.vector.scalar_tensor_tensor(
                            out=nmr[:], in0=mu2[:], scalar=-0.5,
                            in1=rstd[:], op0=mult, op1=mult)

                        halves = []
                        for oc in range(2):
                            kf = a_kvf.tile([128, 512], bf16, tag="kf")
                            vf = a_kvf.tile([128, 512], bf16, tag="vf")
                            kc0 = 4 * oc       # stat column of k head 0
                            vc0 = 8 + 4 * oc   # stat column of v head 0
                            for h in range(4):
                                ph = slice(h * DK, (h + 1) * DK)
                                # x*rstd + nmr; K applies on Scalar, V on DVE
                                nc.scalar.activation(
                                    out=kf[:, ph], in_=kb[oc][:, h, :],
                                    func=ACT.Identity,
                                    bias=nmr[:, kc0 + h:kc0 + h + 1],
                                    scale=rstd[:, kc0 + h:kc0 + h + 1])
                                nc.vector.tensor_scalar(
                                    out=vf[:, ph], in0=vb[oc][:, h, :],
                                    scalar1=rstd[:, vc0 + h:vc0 + h + 1],
                                    scalar2=nmr[:, vc0 + h:vc0 + h + 1],
                                    op0=mult, op1=add_)
                            if has_affine:
                                gb = slice(oc * 512, (oc + 1) * 512)
                                nc.vector.tensor_mul(kf[:], kf[:], gamma_sb[:, gb])
                                nc.vector.tensor_add(kf[:], kf[:], beta_sb[:, gb])
                                nc.vector.tensor_scalar_mul(
                                    out=kf[:], in0=kf[:],
                                    scalar1=delta_sb[:, gsub:gsub + 1])
                                nc.vector.tensor_mul(vf[:], vf[:], gamma_sb[:, gb])
                                nc.vector.tensor_add(vf[:], vf[:], beta_sb[:, gb])
                            halves.append((kf, vf))

                        # Depth-4 deferral: a subtile's scores go to the PE
                        # queue four projection blocks later, so the LN chain
                        # never stalls the PE; items carry across batch
                        # boundaries so the batch tail can't stall either.
                        last = (s == NSUP - 1 and sb == NSUB - 1)
                        if len(pendq) == 4:
                            emit_scores(pendq.pop(0))
                        pendq.append((b, scores_ps, halves, last))
                    if q_in_a:
                        nc.gpsimd.dma_start(out=qt_dr[(b, s)], in_=qt_stage[:])

            while pendq:
                emit_scores(pendq.pop(0))

        # ------- Phase B (fused): qT, attn, x1T, FFN1 -> g, FFN2+bypass ---
        with (
            tc.tile_pool(name="b_q", bufs=2) as b_q,
            tc.tile_pool(name="b_x1", bufs=2) as b_x1,
            tc.tile_pool(name="b_x18", bufs=2) as b_x18,
            tc.tile_pool(name="b_g", bufs=1) as b_g,
            tc.tile_pool(name="b_s", bufs=4) as b_s,
            tc.tile_pool(name="b_y", bufs=2) as b_y,
            tc.tile_pool(name="b_o", bufs=1) as b_o,
            tc.tile_pool(name="b_qps", bufs=2, space="PSUM") as b_qps,
            tc.tile_pool(name="b_aps", bufs=2, space="PSUM") as b_aps,
            tc.tile_pool(name="b_hps", bufs=2, space="PSUM") as b_hps,
            tc.tile_pool(name="b_yps", bufs=2, space="PSUM") as b_yps,
        ):
            for b in range(B):
                # Reduced bf16 scores land directly in their SBUF tile.
                nc.sync.dma_start(out=scores_bf[:, b], in_=cc_out[b])

                for s in range(NSUP):
                    q_in_a = (b, s) in Q_IN_A
                    if b == 0 and s == 0:
                        xtb8 = xtb00
                        xtbb = xtbb00
                    else:
                        if not q_in_a:
                            xtb8 = b_x.tile([128, KC, SUP], f8, tag="xtb1")
                            nc.sync.dma_start(out=xtb8[:], in_=xTb_d.ap()[b, s])
                        xtbb = b_xb.tile([128, KC, SUP], bf16, tag="xtbb")
                        nc.sync.dma_start(out=xtbb[:], in_=xTbBf_d.ap()[b, s])

                    qt = b_q.tile([128, H, SUP], bf16, tag="qt")
                    if q_in_a:
                        # Q for this super-tile was computed during phase A
                        # and staged in DRAM (bf16, bitwise identical).
                        nc.sync.dma_start(out=qt[:], in_=qt_dr[(b, s)])
                    else:
                        for m in range(KC):
                            q_ps = b_qps.tile([128, SUP], f32, tag="qps")
                            for kc in range(0, KC, 2):
                                nc.tensor.matmul(
                                    q_ps[:], lhsT=wq_sb[:, kc:kc + 2, m * 128:(m + 1) * 128],
                                    rhs=xtb8[:, kc:kc + 2, :],
                                    start=(kc == 0), stop=(kc == KC - 2),
                                    perf_mode=DR)
                            nc.scalar.activation(out=qt[:, m, :], in_=q_ps[:],
                                                 func=ACT.Identity, bias=bq_sb[:, m:m + 1],
                                                 scale=1.0 / SW)

                    x1b = b_x1.tile([128, KC, SUP], bf16, tag="x1")
                    x18 = b_x18.tile([128, KC, SUP], f8, tag="x18")
                    for h in range(H):
                        a_ps = b_aps.tile([128, SUP], f32, tag="aps")
                        nc.tensor.matmul(a_ps[:], lhsT=scores_bf[:, b, h, :],
                                         rhs=qt[:, h, :], start=True, stop=True)
                        nc.vector.tensor_add(x1b[:, h, :], a_ps[:], xtbb[:, h, :])
                        nc.scalar.activation(out=x18[:, h, :], in_=x1b[:, h, :],
                                             func=ACT.Copy)

                    g8 = b_g.tile([128, FC, SUP], f8, tag="g8")
                    for m in range(FC):
                        h_ps = b_hps.tile([128, SUP], f32, tag="hps")
                        for kc in range(0, KC, 2):
                            nc.tensor.matmul(
                                h_ps[:], lhsT=w1_sb[:, kc:kc + 2, m * 128:(m + 1) * 128],
                                rhs=x18[:, kc:kc + 2, :],
                                start=(kc == 0), stop=(kc == KC - 2),
                                perf_mode=DR)
                        st = b_s.tile([128, SUP], f32, tag="silu")
                        nc.scalar.activation(out=st[:], in_=h_ps[:], func=ACT.Silu,
                                             bias=b1_sb[:, m:m + 1], scale=1.0 / SW)
                        if has_b1:
                            # g = silu(z) - z/2 with z = ps/SW + b1
                            hz = b_s.tile([128, SUP], f32, tag="hz")
                            nc.scalar.activation(out=hz[:], in_=h_ps[:], func=ACT.Identity,
                                                 bias=b1h_sb[:, m:m + 1], scale=0.5 / SW)
                            nc.vector.scalar_tensor_tensor(
                                out=g8[:, m, :], in0=hz[:], scalar=-1.0,
                                in1=st[:], op0=mult, op1=add_)
                        else:
                            nc.vector.scalar_tensor_tensor(
                                out=g8[:, m, :], in0=h_ps[:], scalar=-0.5 / SW,
                                in1=st[:], op0=mult, op1=add_)

                    ot = b_o.tile([128, KC, SUP], bf16, tag="ot")
                    for m in range(KC):
                        y_ps = b_yps.tile([128, SUP], f32, tag="yps")
                        for kc in range(0, FC, 2):
                            nc.tensor.matmul(
                                y_ps[:], lhsT=w2_sb[:, kc:kc + 2, m * 128:(m + 1) * 128],
                                rhs=g8[:, kc:kc + 2, :],
                                start=(kc == 0), stop=False, perf_mode=DR)
                        # Precise bf16 bypass: + x1 @ (SW*0.5*W1@W2)
                        for kc in range(KC):
                            nc.tensor.matmul(
                                y_ps[:], lhsT=w12_sb[:, kc, m * 128:(m + 1) * 128],
                                rhs=x1b[:, kc, :],
                                start=False, stop=(kc == KC - 1))
                        yt = b_y.tile([128, SUP], f32, tag="yt")
                        if has_b2:
                            nc.scalar.activation(out=yt[:], in_=y_ps[:], func=ACT.Identity,
                                                 bias=b2_sb[:, m:m + 1], scale=1.0 / SW)
                        else:
                            nc.scalar.activation(out=yt[:], in_=y_ps[:], func=ACT.Copy,
                                                 scale=1.0 / SW)
                        nc.vector.tensor_add(ot[:, m, :], yt[:], x1b[:, m, :])
                    # Two half-writes: the first drains while the last four
                    # FFN2 output chunks are still computing.
                    nc.gpsimd.dma_start(out=out_d.ap()[b, s][:, 0:KC // 2, :],
                                        in_=ot[:, 0:KC // 2, :])
                    nc.gpsimd.dma_start(out=out_d.ap()[b, s][:, KC // 2:KC, :],
                                        in_=ot[:, KC // 2:KC, :])

        b_xb_cm.__exit__(None, None, None)
        b_x_cm.__exit__(None, None, None)
        w_ab1_cm.__exit__(None, None, None)
        w_b2a_cm.__exit__(None, None, None)

    nc.finalize()
    return nc


def _get_graph(flags):
    if flags not in _GRAPH_CACHE:
        _GRAPH_CACHE[flags] = _build(flags)
    return _GRAPH_CACHE[flags]


def kernel(x, delta_x, Wq, bq, Wk, bk, Wv, bv, gamma_k, beta_k, W1, b1, W2, b2,
           _trace=False):
    from concourse.bass_utils import run_bass_kernel_spmd

    bf = ml_dtypes.bfloat16
    f8 = ml_dtypes.float8_e4m3
    x = np.asarray(x, np.float32)
    delta_x = np.asarray(delta_x, np.float32)
    Wq, Wk, Wv = (np.asarray(w, np.float32) for w in (Wq, Wk, Wv))
    W1, W2 = np.asarray(W1, np.float32), np.asarray(W2, np.float32)
    bq, bk, bv = (np.asarray(v, np.float32) for v in (bq, bk, bv))
    b1, b2 = np.asarray(b1, np.float32), np.asarray(b2, np.float32)
    gamma_k = np.asarray(gamma_k, np.float32)
    beta_k = np.asarray(beta_k, np.float32)

    has_bk = bool(np.any(bk))
    has_bv = bool(np.any(bv))
    has_b1 = bool(np.any(b1))
    has_b2 = bool(np.any(b2))
    has_affine = not (np.all(gamma_k == 1.0) and np.all(beta_k == 0.0))
    flags = (has_bk, has_bv, has_b1, has_b2, has_affine)
    nc = _get_graph(flags)

    sw = np.float32(SW)

    def wtile(W, nchunks):
        # [nchunks*128, F] -> SBUF layout [128, nchunks, F]
        return np.ascontiguousarray(W.reshape(nchunks, 128, -1).transpose(1, 0, 2))

    wq_8 = wtile((Wq * sw).astype(f8), KC)
    wk_8 = wtile((Wk * sw).astype(f8), KC)
    wv_8 = wtile((Wv * sw).astype(f8), KC)
    w1_8 = wtile((W1 * sw).astype(f8), KC)
    w2_8 = wtile((W2 * sw).astype(f8), FC)
    w12_b = wtile((np.float32(0.5) * sw * (W1 @ W2)).astype(bf), KC)
    bq_t = np.ascontiguousarray(bq.reshape(KC, 128).T)
    b1_t = np.ascontiguousarray(b1.reshape(FC, 128).T)
    delta_pre = (delta_x / np.float32(N)).astype(np.float32)

    in_maps = []
    for c in range(N_CORES):
        t0 = c * NT
        # [B, NT, D] -> [B, NSUP, 128, KC, SUP]: [b,s,p,kc,t] = x[b, s*SUP+t, kc*128+p]
        xT = np.ascontiguousarray(
            x[:, t0:t0 + NT, :].reshape(B, NSUP, SUP, KC, 128).transpose(0, 1, 4, 3, 2))
        m = {"xTb": xT.astype(f8), "xTbBf": xT.astype(bf),
             "delta": np.ascontiguousarray(
                 delta_pre[t0:t0 + NT].reshape(NT // 128, 128).T),
             "Wq": wq_8, "Wk": wk_8, "Wv": wv_8, "W1": w1_8, "W2": w2_8,
             "W12": w12_b, "bq": bq_t, "b1": b1_t}
        if has_b1:
            m["b1h"] = np.ascontiguousarray((b1 * np.float32(0.5)).reshape(FC, 128).T)
        if has_bk:
            m["bk"] = (bk * sw).astype(np.float32)
        if has_bv:
            m["bv"] = (bv * sw).astype(np.float32)
        if has_b2:
            m["b2"] = np.ascontiguousarray(b2.reshape(KC, 128).T)
        if has_affine:
            m["gamma"] = gamma_k.reshape(D).copy()
            m["beta"] = beta_k.reshape(D).copy()
        in_maps.append(m)

    res = run_bass_kernel_spmd(nc, in_maps, core_ids=list(range(N_CORES)),
                               trace=_trace)

    out = np.empty((B, N, D), np.float32)
    for c in range(N_CORES):
        t0 = c * NT
        # [B, NSUP, 128, KC, SUP] -> [B, NT, D]
        ot = res.results[c]["outT"].transpose(0, 1, 4, 3, 2).astype(np.float32)
        out[:, t0:t0 + NT, :] = ot.reshape(B, NT, D)
    if _trace:
        return out, res
    return out


# revision 26
# speedup vs baseline: 1.0210x; 1.0210x over previous
"""Galerkin-attention encoder block on 8 TRN2 NeuronCores.

Sharding: tokens (N=8192 -> 1024/core). The only cross-core dependency is
the Galerkin contraction scores[b,h] = sum_n k[n] (x) v[n] / N, reduced with
four per-batch bf16 AllReduces that overlap local compute.

All device compute runs in "transposed space" (features on partitions,
tokens on the free axis) against host-side pre-transposed x^T, so the
kernel needs no on-device transposes anywhere.

Precision/speed scheme: every large matmul (QKV projections, FFN1, FFN2)
runs in fp8e4m3 with DoubleRow perf mode (2 contraction chunks per
instruction). Weights are pre-scaled by 32 into fp8's normal range; the
1/32 descale folds into the activation-scale of the op that drains each
PSUM accumulation. K/V feed LayerNorm, which is scale-invariant, so their
descale is free. The fp8 noise of the FFN is tamed by routing the linear
bulk of SiLU through a precise bf16 bypass:
  silu(z) = z/2 + g(z),  g(z) = (z/2)*tanh(z/2)
  ffn_out = x1 @ (0.5*W1@W2) [bf16, D x D]  +  g(z) @ W2 [fp8]
W12 = 0.5*W1@W2 is computed host-side; the bypass matmuls accumulate into
the same PSUM group as the fp8 FFN2 matmuls. The residual stream x, x1
stays bf16 end to end.

Schedule notes (vs the first working version):
- Startup: only the tensors the first matmuls need (x super-tile 0, Wk,
  Wv) are DMA'd ahead of the 11MB of phase-B weights, which previously
  monopolized HBM for ~50us before the first matmul could issue.
- Phase A LN runs batched: one segmented bn_stats per K/V projection half
  (4 heads at once) and all mean/var combine arithmetic on [128,16] tiles,
  quartering DVE instruction count (DVE was phase A's bottleneck at 96%).
- The scores AllReduce runs in bf16 (error-neutral, verified in sim) so
  the reduced scores DMA straight into their SBUF tile with no f32
  staging + cast copy.
- The last two subtiles' scores matmuls of each batch are deferred past
  the next batch's first projections, so the batch boundary no longer
  stalls the PE on the LayerNorm tail.
"""

import numpy as np
import ml_dtypes

B, N, D = 4, 8192, 1024
H, DK = 8, 128
FF = 4096
EPS = 1e-5
N_CORES = 8
NT = N // N_CORES          # tokens per core
KC = D // 128              # feature chunks of 128
FC = FF // 128
SUP = 512                  # tokens per super-tile in phase B
NSUP = NT // SUP
SUB = 128                  # tokens per sub-tile in phase A
NSUB = SUP // SUB
SW = 32.0                  # fp8 weight pre-scale

_GRAPH_CACHE = {}


def _build(flags):
    import concourse.bass as bass
    import concourse.tile as tile
    from concourse import bacc, mybir
    from contextlib import ExitStack

    has_bk, has_bv, has_b1, has_b2, has_affine = flags
    f32 = mybir.dt.float32
    bf16 = mybir.dt.bfloat16
    f8 = mybir.dt.float8e4
    DR = mybir.MatmulPerfMode.DoubleRow

    nc = bacc.Bacc("TRN2", target_bir_lowering=False, debug=False,
                   num_devices=N_CORES)

    # All tensors arrive pre-tiled in their exact SBUF layouts so every DMA
    # is a contiguous-per-partition copy.
    xTb_d = nc.dram_tensor("xTb", [B, NSUP, 128, KC, SUP], f8, kind="ExternalInput")
    xTbBf_d = nc.dram_tensor("xTbBf", [B, NSUP, 128, KC, SUP], bf16, kind="ExternalInput")
    delta_d = nc.dram_tensor("delta", [128, NT // 128], f32, kind="ExternalInput")
    wq_d = nc.dram_tensor("Wq", [128, KC, D], f8, kind="ExternalInput")
    wk_d = nc.dram_tensor("Wk", [128, KC, D], f8, kind="ExternalInput")
    wv_d = nc.dram_tensor("Wv", [128, KC, D], f8, kind="ExternalInput")
    w1_d = nc.dram_tensor("W1", [128, KC, FF], f8, kind="ExternalInput")
    w2_d = nc.dram_tensor("W2", [128, FC, D], f8, kind="ExternalInput")
    w12_d = nc.dram_tensor("W12", [128, KC, D], bf16, kind="ExternalInput")
    bq_d = nc.dram_tensor("bq", [128, KC], f32, kind="ExternalInput")
    b1_d = nc.dram_tensor("b1", [128, FC], f32, kind="ExternalInput")
    b1h_d = nc.dram_tensor("b1h", [128, FC], f32, kind="ExternalInput") if has_b1 else None
    bk_d = nc.dram_tensor("bk", [D], f32, kind="ExternalInput") if has_bk else None
    bv_d = nc.dram_tensor("bv", [D], f32, kind="ExternalInput") if has_bv else None
    b2_d = nc.dram_tensor("b2", [128, KC], f32, kind="ExternalInput") if has_b2 else None
    gamma_d = nc.dram_tensor("gamma", [D], f32, kind="ExternalInput") if has_affine else None
    beta_d = nc.dram_tensor("beta", [D], f32, kind="ExternalInput") if has_affine else None
    out_d = nc.dram_tensor("outT", [B, NSUP, 128, KC, SUP], bf16, kind="ExternalOutput")

    # Super-tiles whose Q projection runs inside phase A (filling its PE
    # bubbles) with the result staged through a DRAM scratch. (0,0) stays in
    # phase B: its window races the Wq load at startup.
    Q_IN_A = set()  # empty: adding PE work to the DVE-bound phase A was a net loss

    sub_ = mybir.AluOpType.subtract
    mult = mybir.AluOpType.mult
    add_ = mybir.AluOpType.add
    ACT = mybir.ActivationFunctionType

    with tile.TileContext(nc) as tc, ExitStack() as ctx:
        singles = ctx.enter_context(tc.tile_pool(name="singles", bufs=1))
        dram = ctx.enter_context(tc.tile_pool(name="dram", bufs=1, space="DRAM"))

        eps_t = singles.tile([128, 1], f32)
        nc.vector.memset(eps_t, EPS)
        delta_sb = singles.tile([128, NT // 128], f32)
        scores_bf = singles.tile([128, B, H, DK], bf16)

        cc_in = dram.tile([B, 128, H, DK], bf16)
        cc_out = [dram.tile([128, H, DK], bf16, addr_space="Shared",
                            name=f"cc_out{b}") for b in range(B)]

        # Weight pools span both phases; DMAs are issued inside phase A
        # AFTER the startup-critical loads so they don't starve them.
        w_b2a_cm = tc.tile_pool(name="w_b2a", bufs=1)
        w_b2a = w_b2a_cm.__enter__()
        w2_sb = w_b2a.tile([128, FC, D], f8)
        w12_sb = w_b2a.tile([128, KC, D], bf16)
        w_ab1_cm = tc.tile_pool(name="w_ab1", bufs=1)
        w_ab1 = w_ab1_cm.__enter__()
        wq_sb = w_ab1.tile([128, KC, D], f8)
        w1_sb = w_ab1.tile([128, KC, FF], f8)
        bq_sb = w_ab1.tile([128, KC], f32)
        b1_sb = w_ab1.tile([128, FC], f32)

        # Phase-B x pools opened early so (b=0, s=0) can prefetch during
        # phase A; the fp8 tile doubles as phase A's first x super-tile.
        b_x_cm = tc.tile_pool(name="b_x", bufs=2)
        b_x = b_x_cm.__enter__()
        b_xb_cm = tc.tile_pool(name="b_xb", bufs=2)
        b_xb = b_xb_cm.__enter__()

        # ---------------- Phase A: k, v, LN, partial scores, AllReduce ----
        with (
            tc.tile_pool(name="wa", bufs=1) as wa,
            tc.tile_pool(name="a_x", bufs=2) as a_x,
            tc.tile_pool(name="a_kvf", bufs=10) as a_kvf,
            tc.tile_pool(name="a_kvb", bufs=4) as a_kvb,
            tc.tile_pool(name="a_ln", bufs=8) as a_ln,
            tc.tile_pool(name="a_sc", bufs=2) as a_sc,
            tc.tile_pool(name="a_qt", bufs=2) as a_qt,
            tc.tile_pool(name="a_kvps", bufs=6, space="PSUM") as a_kvps,
            tc.tile_pool(name="a_sps", bufs=1, space="PSUM") as a_sps,
            tc.tile_pool(name="a_qps", bufs=1, space="PSUM") as a_qps,
        ):
            qt_dr = {bs: dram.tile([128, KC, SUP], bf16, name=f"qtd{bs[0]}{bs[1]}")
                     for bs in Q_IN_A}
            # Startup-critical DMAs first, all on the sync queue except Wv
            # (gpsimd) so the two halves land in parallel.
            xtb00 = b_x.tile([128, KC, SUP], f8, tag="xtb1")
            nc.sync.dma_start(out=xtb00[:], in_=xTb_d.ap()[0, 0])
            wk_sb = wa.tile([128, KC, D], f8)
            nc.sync.dma_start(out=wk_sb[:, :, 0:512], in_=wk_d.ap()[:, :, 0:512])
            nc.sync.dma_start(out=wk_sb[:, :, 512:D], in_=wk_d.ap()[:, :, 512:D])
            wv_sb = wa.tile([128, KC, D], f8)
            nc.gpsimd.dma_start(out=wv_sb[:, :, 0:512], in_=wv_d.ap()[:, :, 0:512])
            nc.gpsimd.dma_start(out=wv_sb[:, :, 512:D], in_=wv_d.ap()[:, :, 512:D])
            nc.sync.dma_start(out=delta_sb[:], in_=delta_d.ap())
            xtbb00 = b_xb.tile([128, KC, SUP], bf16, tag="xtbb")
            nc.sync.dma_start(out=xtbb00[:], in_=xTbBf_d.ap()[0, 0])
            # Bulk phase-B weights stream behind the critical loads on the
            # same (sync) queue; phase A x tiles ride gpsimd so they never
            # queue behind this 11MB.
            nc.sync.dma_start(out=wq_sb[:], in_=wq_d.ap())
            nc.sync.dma_start(out=w1_sb[:], in_=w1_d.ap())
            nc.sync.dma_start(out=w2_sb[:], in_=w2_d.ap())
            nc.sync.dma_start(out=w12_sb[:], in_=w12_d.ap())
            nc.scalar.dma_start(out=bq_sb[:], in_=bq_d.ap())
            nc.scalar.dma_start(out=b1_sb[:], in_=b1_d.ap())
            if has_b1:
                b1h_sb = w_ab1.tile([128, FC], f32)
                nc.scalar.dma_start(out=b1h_sb[:], in_=b1h_d.ap())
            if has_b2:
                b2_sb = w_ab1.tile([128, KC], f32)
                nc.scalar.dma_start(out=b2_sb[:], in_=b2_d.ap())
            if has_affine:
                gamma_sb = w_ab1.tile([128, D], f32)
                nc.scalar.dma_start(out=gamma_sb[:], in_=gamma_d.ap().to_broadcast([128, D]))
                beta_sb = w_ab1.tile([128, D], f32)
                nc.scalar.dma_start(out=beta_sb[:], in_=beta_d.ap().to_broadcast([128, D]))
            if has_bk:
                bk_sb = w_ab1.tile([128, 2, 4, 128], f32)
                nc.scalar.dma_start(out=bk_sb[:], in_=bk_d.ap().to_broadcast([128, D]))
            if has_bv:
                bv_sb = w_ab1.tile([128, 2, 4, 128], f32)
                nc.scalar.dma_start(out=bv_sb[:], in_=bv_d.ap().to_broadcast([128, D]))

            def drain_scores(b, scores_ps):
                # f32 PSUM -> bf16 staging -> HBM -> bf16 AllReduce. The
                # reduced scores later DMA straight into scores_bf.
                sc_sb = a_sc.tile([128, H, DK], bf16, tag="scsb")
                nc.scalar.activation(out=sc_sb[:], in_=scores_ps[:, :, :],
                                     func=ACT.Copy)
                nc.sync.dma_start(out=cc_in[b], in_=sc_sb[:])
                nc.gpsimd.collective_compute(
                    "AllReduce", mybir.AluOpType.add,
                    replica_groups=[list(range(N_CORES))],
                    ins=[cc_in[b].opt()], outs=[cc_out[b].opt()])

            def emit_scores(item):
                b, scores_ps, halves, last = item
                for oc in range(2):
                    kf, vf = halves[oc]
                    for h in range(4):
                        ph = slice(h * DK, (h + 1) * DK)
                        nc.tensor.matmul(
                            scores_ps[:, oc * 4 + h, :], lhsT=kf[:, ph], rhs=vf[:, ph],
                            start=False, stop=last, skip_group_check=True)
                if last:
                    drain_scores(b, scores_ps)

            pendq = []
            for b in range(B):
                scores_ps = a_sps.tile([128, H, DK], f32, tag="scores")
                # Four per-head accumulation groups share each PSUM bank, and
                # a start=True matmul clears its whole bank - zero once and
                # accumulate with start=False instead.
                nc.vector.memset(scores_ps, 0.0)

                for s in range(NSUP):
                    if b == 0 and s == 0:
                        xtb = xtb00
                    else:
                        xtb = a_x.tile([128, KC, SUP], f8, tag="xtb")
                        nc.gpsimd.dma_start(out=xtb[:], in_=xTb_d.ap()[b, s])
                    q_in_a = (b, s) in Q_IN_A
                    if q_in_a:
                        qt_stage = a_qt.tile([128, KC, SUP], bf16, tag="qstage")

                    def emit_q(qm):
                        # One Q output chunk for this super-tile, filling PE
                        # bubbles left by the DVE-bound LN chain. Numerics
                        # are identical to the phase-B version.
                        q_ps = a_qps.tile([128, SUP], f32, tag="qps")
                        for kc in range(0, KC, 2):
                            nc.tensor.matmul(
                                q_ps[:],
                                lhsT=wq_sb[:, kc:kc + 2, qm * 128:(qm + 1) * 128],
                                rhs=xtb[:, kc:kc + 2, :],
                                start=(kc == 0), stop=(kc == KC - 2),
                                perf_mode=DR)
                        nc.scalar.activation(out=qt_stage[:, qm, :], in_=q_ps[:],
                                             func=ACT.Identity,
                                             bias=bq_sb[:, qm:qm + 1],
                                             scale=1.0 / SW)

                    for sb in range(NSUB):
                        gsub = s * NSUB + sb
                        tsl = bass.ts(sb, SUB)

                        def half_proj(w_sb, bias_sb, oc, tag):
                            # One 512-feature half of a K/V projection,
                            # shaped [tok, head, dk] for segmented bn_stats.
                            ps = a_kvps.tile([128, 4, DK], f32, tag="kv")
                            for kc in range(0, KC, 2):
                                nc.tensor.matmul(
                                    ps[:, :, :],
                                    lhsT=xtb[:, kc:kc + 2, tsl],
                                    rhs=w_sb[:, kc:kc + 2, oc * 512:(oc + 1) * 512],
                                    start=(kc == 0), stop=(kc == KC - 2),
                                    perf_mode=DR)
                            if bias_sb is not None:
                                nc.vector.tensor_add(ps[:, :, :], ps[:, :, :],
                                                     bias_sb[:, oc])
                            return ps

                        k_ps = [half_proj(wk_sb, bk_sb if has_bk else None, oc, "k")
                                for oc in range(2)]
                        if q_in_a:
                            emit_q(2 * sb)
                        v_ps = [half_proj(wv_sb, bv_sb if has_bv else None, oc, "v")
                                for oc in range(2)]
                        if q_in_a:
                            emit_q(2 * sb + 1)

                        # Drain PSUM to SBUF bf16 immediately: the PSUM banks
                        # recycle after one op instead of after the whole LN
                        # chain, so the PE never stalls on bank pressure (a
                        # stall >3.4us re-throttles the PE clock). LN then
                        # runs entirely from SBUF at 16-bit DVE speed.
                        kb = []
                        vb = []
                        for oc in range(2):
                            kbt = a_kvb.tile([128, 4, DK], bf16, tag="kb")
                            nc.scalar.activation(out=kbt[:, :, :],
                                                 in_=k_ps[oc][:, :, :],
                                                 func=ACT.Copy)
                            kb.append(kbt)
                            vbt = a_kvb.tile([128, 4, DK], bf16, tag="vb")
                            nc.vector.tensor_copy(out=vbt[:, :, :],
                                                  in_=v_ps[oc][:, :, :])
                            vb.append(vbt)

                        # Per-head stats into one batched tile; stat columns
                        # are [k0 h0-3 | k1 h0-3 | v0 h0-3 | v1 h0-3] so the
                        # delta fold below is a single [128,8] op.
                        stats = a_ln.tile([128, 16, 6], f32, tag="stats")
                        for oc in range(2):
                            for h in range(4):
                                nc.vector.bn_stats(out=stats[:, 4 * oc + h, :],
                                                   in_=kb[oc][:, h, :])
                                nc.vector.bn_stats(out=stats[:, 8 + 4 * oc + h, :],
                                                   in_=vb[oc][:, h, :])
                        # Combine bn_stats' even/odd halves with batched
                        # [128,16] arithmetic:
                        # M2 = M2e + M2o + (mu_e - mu_o)^2 * (DK/4),
                        # var = M2/DK, mu2 = mu_e + mu_o (= 2*mu).
                        dmu = a_ln.tile([128, 16], f32, tag="dmu")
                        nc.vector.scalar_tensor_tensor(
                            out=dmu[:], in0=stats[:, :, 1], scalar=1.0,
                            in1=stats[:, :, 4], op0=mult, op1=sub_)
                        m2 = a_ln.tile([128, 16], f32, tag="m2")
                        nc.vector.scalar_tensor_tensor(
                            out=m2[:], in0=stats[:, :, 2], scalar=1.0,
                            in1=stats[:, :, 5], op0=mult, op1=add_)
                        dmu2 = a_ln.tile([128, 16], f32, tag="dmu2")
                        nc.vector.tensor_mul(dmu2[:], dmu[:], dmu[:])
                        m2t = a_ln.tile([128, 16], f32, tag="m2t")
                        nc.vector.scalar_tensor_tensor(
                            out=m2t[:], in0=dmu2[:], scalar=float(DK) / 4.0,
                            in1=m2[:], op0=mult, op1=add_)
                        mu2 = a_ln.tile([128, 16], f32, tag="mu2")
                        nc.vector.scalar_tensor_tensor(
                            out=mu2[:], in0=stats[:, :, 1], scalar=1.0,
                            in1=stats[:, :, 4], op0=mult, op1=add_)
                        # rstd = 1/sqrt(var+eps) in one ACT (var >= 0 so the
                        # abs is a no-op; fp8 noise dwarfs the table error)
                        rstd = a_ln.tile([128, 16], f32, tag="rstd")
                        nc.scalar.activation(out=rstd[:], in_=m2t[:],
                                             func=ACT.Abs_reciprocal_sqrt,
                                             bias=eps_t[:], scale=1.0 / DK)
                        if not has_affine:
                            # delta/N folds into K's rstd (columns 0-7).
                            # (stays on DVE: Pool lacks TensorScalarPtr)
                            nc.vector.tensor_scalar_mul(
                                out=rstd[:, 0:8], in0=rstd[:, 0:8],
                                scalar1=delta_sb[:, gsub:gsub + 1])
                        # nmr = -mu * rstd  (= (mu2 * -0.5) * rstd)
                        nmr = a_ln.tile([128, 16], f32, tag="nmr")
                        nc.vector.scalar_tensor_tensor(
                            out=nmr[:], in0=mu2[:], scalar=-0.5,
                            in1=rstd[:], op0=mult, op1=mult)

                        halves = []
                        for oc in range(2):
                            kf = a_kvf.tile([128, 512], bf16, tag="kf")
                            vf = a_kvf.tile([128, 512], bf16, tag="vf")
                            kc0 = 4 * oc       # stat column of k head 0
                            vc0 = 8 + 4 * oc   # stat column of v head 0
                            for h in range(4):
                                ph = slice(h * DK, (h + 1) * DK)
                                # x*rstd + nmr; K applies on Scalar, V split
                                # 5 DVE / 3 Scalar to balance phase A load.
                                nc.scalar.activation(
                                    out=kf[:, ph], in_=kb[oc][:, h, :],
                                    func=ACT.Identity,
                                    bias=nmr[:, kc0 + h:kc0 + h + 1],
                                    scale=rstd[:, kc0 + h:kc0 + h + 1])
                                if oc == 1 and h >= 1:
                                    nc.scalar.activation(
                                        out=vf[:, ph], in_=vb[oc][:, h, :],
                                        func=ACT.Identity,
                                        bias=nmr[:, vc0 + h:vc0 + h + 1],
                                        scale=rstd[:, vc0 + h:vc0 + h + 1])
                                else:
                                    nc.vector.tensor_scalar(
                                        out=vf[:, ph], in0=vb[oc][:, h, :],
                                        scalar1=rstd[:, vc0 + h:vc0 + h + 1],
                                        scalar2=nmr[:, vc0 + h:vc0 + h + 1],
                                        op0=mult, op1=add_)
                            if has_affine:
                                gb = slice(oc * 512, (oc + 1) * 512)
                                nc.vector.tensor_mul(kf[:], kf[:], gamma_sb[:, gb])
                                nc.vector.tensor_add(kf[:], kf[:], beta_sb[:, gb])
                                nc.vector.tensor_scalar_mul(
                                    out=kf[:], in0=kf[:],
                                    scalar1=delta_sb[:, gsub:gsub + 1])
                                nc.vector.tensor_mul(vf[:], vf[:], gamma_sb[:, gb])
                                nc.vector.tensor_add(vf[:], vf[:], beta_sb[:, gb])
                            halves.append((kf, vf))

                        # Depth-4 deferral: a subtile's scores go to the PE
                        # queue four projection blocks later, so the LN chain
                        # never stalls the PE; items carry across batch
                        # boundaries so the batch tail can't stall either.
                        last = (s == NSUP - 1 and sb == NSUB - 1)
                        if len(pendq) == 4:
                            emit_scores(pendq.pop(0))
                        pendq.append((b, scores_ps, halves, last))
                    if q_in_a:
                        nc.gpsimd.dma_start(out=qt_dr[(b, s)], in_=qt_stage[:])

            while pendq:
                emit_scores(pendq.pop(0))

        # ------- Phase B (fused): qT, attn, x1T, FFN1 -> g, FFN2+bypass ---
        with (
            tc.tile_pool(name="b_q", bufs=2) as b_q,
            tc.tile_pool(name="b_x1", bufs=2) as b_x1,
            tc.tile_pool(name="b_x18", bufs=2) as b_x18,
            tc.tile_pool(name="b_g", bufs=1) as b_g,
            tc.tile_pool(name="b_s", bufs=4) as b_s,
            tc.tile_pool(name="b_y", bufs=2) as b_y,
            tc.tile_pool(name="b_o", bufs=1) as b_o,
            tc.tile_pool(name="b_qps", bufs=2, space="PSUM") as b_qps,
            tc.tile_pool(name="b_aps", bufs=2, space="PSUM") as b_aps,
            tc.tile_pool(name="b_hps", bufs=2, space="PSUM") as b_hps,
            tc.tile_pool(name="b_yps", bufs=2, space="PSUM") as b_yps,
        ):
            for b in range(B):
                # Reduced bf16 scores land directly in their SBUF tile.
                nc.sync.dma_start(out=scores_bf[:, b], in_=cc_out[b])

                for s in range(NSUP):
                    q_in_a = (b, s) in Q_IN_A
                    if b == 0 and s == 0:
                        xtb8 = xtb00
                        xtbb = xtbb00
                    else:
                        if not q_in_a:
                            xtb8 = b_x.tile([128, KC, SUP], f8, tag="xtb1")
                            nc.sync.dma_start(out=xtb8[:], in_=xTb_d.ap()[b, s])
                        xtbb = b_xb.tile([128, KC, SUP], bf16, tag="xtbb")
                        nc.sync.dma_start(out=xtbb[:], in_=xTbBf_d.ap()[b, s])

                    qt = b_q.tile([128, H, SUP], bf16, tag="qt")
                    if q_in_a:
                        # Q for this super-tile was computed during phase A
                        # and staged in DRAM (bf16, bitwise identical).
                        nc.sync.dma_start(out=qt[:], in_=qt_dr[(b, s)])
                    else:
                        for m in range(KC):
                            q_ps = b_qps.tile([128, SUP], f32, tag="qps")
                            for kc in range(0, KC, 2):
                                nc.tensor.matmul(
                                    q_ps[:], lhsT=wq_sb[:, kc:kc + 2, m * 128:(m + 1) * 128],
                                    rhs=xtb8[:, kc:kc + 2, :],
                                    start=(kc == 0), stop=(kc == KC - 2),
                                    perf_mode=DR)
                            nc.scalar.activation(out=qt[:, m, :], in_=q_ps[:],
                                                 func=ACT.Identity, bias=bq_sb[:, m:m + 1],
                                                 scale=1.0 / SW)

                    x1b = b_x1.tile([128, KC, SUP], bf16, tag="x1")
                    x18 = b_x18.tile([128, KC, SUP], f8, tag="x18")
                    for h in range(H):
                        a_ps = b_aps.tile([128, SUP], f32, tag="aps")
                        nc.tensor.matmul(a_ps[:], lhsT=scores_bf[:, b, h, :],
                                         rhs=qt[:, h, :], start=True, stop=True)
                        nc.vector.tensor_add(x1b[:, h, :], a_ps[:], xtbb[:, h, :])
                        nc.scalar.activation(out=x18[:, h, :], in_=x1b[:, h, :],
                                             func=ACT.Copy)

                    g8 = b_g.tile([128, FC, SUP], f8, tag="g8")
                    for m in range(FC):
                        h_ps = b_hps.tile([128, SUP], f32, tag="hps")
                        for kc in range(0, KC, 2):
                            nc.tensor.matmul(
                                h_ps[:], lhsT=w1_sb[:, kc:kc + 2, m * 128:(m + 1) * 128],
                                rhs=x18[:, kc:kc + 2, :],
                                start=(kc == 0), stop=(kc == KC - 2),
                                perf_mode=DR)
                        st = b_s.tile([128, SUP], f32, tag="silu")
                        nc.scalar.activation(out=st[:], in_=h_ps[:], func=ACT.Silu,
                                             bias=b1_sb[:, m:m + 1], scale=1.0 / SW)
                        if has_b1:
                            # g = silu(z) - z/2 with z = ps/SW + b1
                            hz = b_s.tile([128, SUP], f32, tag="hz")
                            nc.scalar.activation(out=hz[:], in_=h_ps[:], func=ACT.Identity,
                                                 bias=b1h_sb[:, m:m + 1], scale=0.5 / SW)
                            nc.vector.scalar_tensor_tensor(
                                out=g8[:, m, :], in0=hz[:], scalar=-1.0,
                                in1=st[:], op0=mult, op1=add_)
                        else:
                            nc.vector.scalar_tensor_tensor(
                                out=g8[:, m, :], in0=h_ps[:], scalar=-0.5 / SW,
                                in1=st[:], op0=mult, op1=add_)

                    ot = b_o.tile([128, KC, SUP], bf16, tag="ot")
                    for m in range(KC):
                        y_ps = b_yps.tile([128, SUP], f32, tag="yps")
                        for kc in range(0, FC, 2):
                            nc.tensor.matmul(
                                y_ps[:], lhsT=w2_sb[:, kc:kc + 2, m * 128:(m + 1) * 128],
                                rhs=g8[:, kc:kc + 2, :],
                                start=(kc == 0), stop=False, perf_mode=DR)
                        # Precise bf16 bypass: + x1 @ (SW*0.5*W1@W2)
                        for kc in range(KC):
                            nc.tensor.matmul(
                                y_ps[:], lhsT=w12_sb[:, kc, m * 128:(m + 1) * 128],
                                rhs=x1b[:, kc, :],
                                start=False, stop=(kc == KC - 1))
                        yt = b_y.tile([128, SUP], f32, tag="yt")
                        if has_b2:
                            nc.scalar.activation(out=yt[:], in_=y_ps[:], func=ACT.Identity,
                                                 bias=b2_sb[:, m:m + 1], scale=1.0 / SW)
                        else:
                            nc.scalar.activation(out=yt[:], in_=y_ps[:], func=ACT.Copy,
                                                 scale=1.0 / SW)
                        nc.vector.tensor_add(ot[:, m, :], yt[:], x1b[:, m, :])
                    # Two half-writes: the first drains while the last four
                    # FFN2 output chunks are still computing.
                    nc.gpsimd.dma_start(out=out_d.ap()[b, s][:, 0:KC // 2, :],
                                        in_=ot[:, 0:KC // 2, :])
                    nc.gpsimd.dma_start(out=out_d.ap()[b, s][:, KC // 2:KC, :],
                                        in_=ot[:, KC // 2:KC, :])

        b_xb_cm.__exit__(None, None, None)
        b_x_cm.__exit__(None, None, None)
        w_ab1_cm.__exit__(None, None, None)
        w_b2a_cm.__exit__(None, None, None)

    nc.finalize()
    return nc


def _get_graph(flags):
    if flags not in _GRAPH_CACHE:
        _GRAPH_CACHE[flags] = _build(flags)
    return _GRAPH_CACHE[flags]


def kernel(x, delta_x, Wq, bq, Wk, bk, Wv, bv, gamma_k, beta_k, W1, b1, W2, b2,
           _trace=False):
    from concourse.bass_utils import run_bass_kernel_spmd

    bf = ml_dtypes.bfloat16
    f8 = ml_dtypes.float8_e4m3
    x = np.asarray(x, np.float32)
    delta_x = np.asarray(delta_x, np.float32)
    Wq, Wk, Wv = (np.asarray(w, np.float32) for w in (Wq, Wk, Wv))
    W1, W2 = np.asarray(W1, np.float32), np.asarray(W2, np.float32)
    bq, bk, bv = (np.asarray(v, np.float32) for v in (bq, bk, bv))
    b1, b2 = np.asarray(b1, np.float32), np.asarray(b2, np.float32)
    gamma_k = np.asarray(gamma_k, np.float32)
    beta_k = np.asarray(beta_k, np.float32)

    has_bk = bool(np.any(bk))
    has_bv = bool(np.any(bv))
    has_b1 = bool(np.any(b1))
    has_b2 = bool(np.any(b2))
    has_affine = not (np.all(gamma_k == 1.0) and np.all(beta_k == 0.0))
    flags = (has_bk, has_bv, has_b1, has_b2, has_affine)
    nc = _get_graph(flags)

    sw = np.float32(SW)

    def wtile(W, nchunks):
        # [nchunks*128, F] -> SBUF layout [128, nchunks, F]
        return np.ascontiguousarray(W.reshape(nchunks, 128, -1).transpose(1, 0, 2))

    wq_8 = wtile((Wq * sw).astype(f8), KC)
    wk_8 = wtile((Wk * sw).astype(f8), KC)
    wv_8 = wtile((Wv * sw).astype(f8), KC)
    w1_8 = wtile((W1 * sw).astype(f8), KC)
    w2_8 = wtile((W2 * sw).astype(f8), FC)
    w12_b = wtile((np.float32(0.5) * sw * (W1 @ W2)).astype(bf), KC)
    bq_t = np.ascontiguousarray(bq.reshape(KC, 128).T)
    b1_t = np.ascontiguousarray(b1.reshape(FC, 128).T)
    delta_pre = (delta_x / np.float32(N)).astype(np.float32)

    in_maps = []
    for c in range(N_CORES):
        t0 = c * NT
        # [B, NT, D] -> [B, NSUP, 128, KC, SUP]: [b,s,p,kc,t] = x[b, s*SUP+t, kc*128+p]
        xT = np.ascontiguousarray(
            x[:, t0:t0 + NT, :].reshape(B, NSUP, SUP, KC, 128).transpose(0, 1, 4, 3, 2))
        m = {"xTb": xT.astype(f8), "xTbBf": xT.astype(bf),
             "delta": np.ascontiguousarray(
                 delta_pre[t0:t0 + NT].reshape(NT // 128, 128).T),
             "Wq": wq_8, "Wk": wk_8, "Wv": wv_8, "W1": w1_8, "W2": w2_8,
             "W12": w12_b, "bq": bq_t, "b1": b1_t}
        if has_b1:
            m["b1h"] = np.ascontiguousarray((b1 * np.float32(0.5)).reshape(FC, 128).T)
        if has_bk:
            m["bk"] = (bk * sw).astype(np.float32)
        if has_bv:
            m["bv"] = (bv * sw).astype(np.float32)
        if has_b2:
            m["b2"] = np.ascontiguousarray(b2.reshape(KC, 128).T)
        if has_affine:
            m["gamma"] = gamma_k.reshape(D).copy()
            m["beta"] = beta_k.reshape(D).copy()
        in_maps.append(m)

    res = run_bass_kernel_spmd(nc, in_maps, core_ids=list(range(N_CORES)),
                               trace=_trace)

    out = np.empty((B, N, D), np.float32)
    for c in range(N_CORES):
        t0 = c * NT
        # [B, NSUP, 128, KC, SUP] -> [B, NT, D]
        ot = res.results[c]["outT"].transpose(0, 1, 4, 3, 2).astype(np.float32)
        out[:, t0:t0 + NT, :] = ot.reshape(B, NT, D)
    if _trace:
        return out, res
    return out
